# revision 48
# baseline (speedup 1.0000x reference)
"""SlimMambaBlock Trainium2 kernel.

Full-input contract: kernel(**inputs) takes the complete tensors
(x [8, 4096, 256], norm_w [256], W_in [1024, 256], W_dt [512, 512],
b_dt [512], W_out [256, 512]) and returns the full output [8, 4096, 256].

Sharding: data-parallel over batch — core b processes batch b (8 cores).

Per-core program (Tile framework), feature-major activation layout:
  1. RMSNorm stats: vpk = sum(x^2) per token via ScalarE Square+accum;
     inv_rms via Newton-rsqrt on GpSimd/DVE (bit-trick seed + 3 iters) so
     the ScalarE only ever needs ONE activation table set
     (silu_and_others; sigmoid/rsqrt would each force a 1.3us reload).
  2. hT = x^T * diag(inv_rms): the normalization is folded into the PE
     transpose as a REGULAR matmul out[d,tok] = sum_p x[p,d]*diag[p,tok]
     (x-tile stationary, diag moving; diag built by GpSimd from the
     identity) — no separate h=x*rms elementwise pass. GpSimd also makes
     a bf16 copy of x so the transpose runs at 1 cycle/row.
  3. in_proj: uvT[feat, tok] = W_inT.T @ hT ; u = silu, g = silu (ACT)
  4. dt_proj: preT = W_dtT.T @ uT ; th = tanh(pre/2 + b_dt/2) (ACT)
     (sigmoid is NOT in the silu activation table set, tanh is)
     lam = sigmoid(pre + b_dt) = 0.5*th + 0.5 (DVE tensor_scalar 4x)
  5. recurrence via tensor_tensor_scan along the time (free) axis, with
     S = -2*s:  S_t = lam_t * S_{t-1} + bT_t,  bT = (th-1)*u computed as
     ONE fused scalar_tensor_tensor.
  6. sg = s*g = (S * -0.5) * g  (fused scalar_tensor_tensor)
  7. out_proj with sgT as the stationary operand: y[tok, d] = sgT.T @ W_outT
  8. the residual is a 5th accumulation matmul into the same PSUM group
     (identity stationary, bf16 x moving), so PSUM holds x+y; ScalarE/DVE
     copies drain it to SBUF and paired DMAs store it.

Matmul operands are bf16 (PE 1 cycle/row vs 4 for fp32; float32r is
rejected by walrus on non-PE engines). RMS stats, Newton-rsqrt, the PE
diag scaling and the final residual add stay fp32; the scan's internal
state is fp32. Measured rel err ~2e-3 (gate 2e-2).

Engine placement of the streaming elementwise work is configurable
(bt_on_pool / sg_on_pool / scan_on_pool) to balance DVE vs GpSimd —
defaults picked from hardware timing.
"""

import numpy as np

B, K, D = 8, 4096, 256
INNER = 512
EPS = 1e-5
TC = 512                 # tokens per chunk
NCHUNK = K // TC         # 8
NTT = TC // 128          # token-tiles per chunk

N_CORES = 8
MAGIC = 0x5F3759DF       # fast inverse sqrt seed

_CACHE: dict = {}


def _emit(tc, aps, mm_f32r=True, silu_native=True, repeat=1,
          bt_on_pool=False, sg_on_pool=False, scan_on_pool=0):
    """Emit the per-core program. aps: dict of DRAM APs."""
    import concourse.bass as bass
    import concourse.mybir as mybir
    from concourse import masks

    nc = tc.nc
    f32 = mybir.dt.float32
    i32 = mybir.dt.int32
    AF = mybir.ActivationFunctionType
    ALU = mybir.AluOpType
    ts = bass.ts

    # Fast mode: all matmul operands in bf16 (PE runs 1 cycle/row vs 4 for
    # fp32; bf16 is encodable on every engine, unlike float32r which walrus
    # rejects outside the PE). The recurrence runs in bf16 tensors with the
    # scan's fp32 internal state.
    fr = mybir.dt.bfloat16 if mm_f32r else f32

    x_d = aps["x"]
    nw_d = aps["norm_w"]
    win_d = aps["W_in"]
    wdt_d = aps["W_dt"]
    bdt_d = aps["b_dt"]
    wout_d = aps["W_out"]
    out_d = aps["out"]

    import contextlib
    ctx = contextlib.ExitStack()
    with ctx:
        const = ctx.enter_context(tc.tile_pool(name="const", bufs=1))
        wraw = ctx.enter_context(tc.tile_pool(name="wraw", bufs=1))
        wT = ctx.enter_context(tc.tile_pool(name="wT", bufs=1))
        xp = ctx.enter_context(tc.tile_pool(name="xp", bufs=2))
        xbp = ctx.enter_context(tc.tile_pool(name="xbp", bufs=3 * NTT))
        sqp = ctx.enter_context(tc.tile_pool(name="sqp", bufs=2))
        statp = ctx.enter_context(tc.tile_pool(name="statp", bufs=6))
        diagp = ctx.enter_context(tc.tile_pool(name="diagp", bufs=2 * NTT))
        hTp = ctx.enter_context(tc.tile_pool(name="hTp", bufs=3))
        uTp = ctx.enter_context(tc.tile_pool(name="uTp", bufs=3))
        gTp = ctx.enter_context(tc.tile_pool(name="gTp", bufs=3))
        thTp = ctx.enter_context(tc.tile_pool(name="thTp", bufs=3))
        lamTp = ctx.enter_context(tc.tile_pool(name="lamTp", bufs=3))
        bTp = ctx.enter_context(tc.tile_pool(name="bTp", bufs=3))
        sTp = ctx.enter_context(tc.tile_pool(name="sTp", bufs=3))
        sgTp = ctx.enter_context(tc.tile_pool(name="sgTp", bufs=3))
        outp = ctx.enter_context(tc.tile_pool(name="outp", bufs=4))

        # PSUM bank budget (8 x 2KB): tps 1 + uvps 3 + preps 2 + yps 2.
        # Deep uvps keeps the PE streaming in_proj while the ScalarE
        # drains earlier banks — PE pstate ramps only when continuously
        # busy (>3us), which halves matmul cycle time.
        tps = ctx.enter_context(tc.tile_pool(name="tps", bufs=1, space="PSUM"))
        uvps = ctx.enter_context(tc.tile_pool(name="uvps", bufs=3, space="PSUM"))
        preps = ctx.enter_context(tc.tile_pool(name="preps", bufs=2, space="PSUM"))
        yps = ctx.enter_context(tc.tile_pool(name="yps", bufs=2, space="PSUM"))

        # ---- constants ----
        identf = const.tile([128, 128], f32, tag="identf", name="identf")
        masks.make_identity(nc, identf[:])
        identb = const.tile([128, 128], fr, tag="identb", name="identb")
        nc.gpsimd.tensor_copy(identb[:], identf[:])
        magic = const.tile([128, NTT], i32, tag="magic", name="magic")
        nc.gpsimd.memset(magic[:], MAGIC)

        nwt = const.tile([128, 2], f32, tag="nw", name="nw")
        nc.sync.dma_start(nwt[:], nw_d.rearrange("(k p) -> p k", p=128))
        nw = [nwt[:, k:k + 1] for k in range(2)]
        bdtt = const.tile([128, 4], f32, tag="bdt", name="bdt")
        nc.sync.dma_start(bdtt[:], bdt_d.rearrange("(m p) -> p m", p=128))
        # scale in place: tanh(pre*0.5 + b_dt*0.5)
        nc.vector.tensor_scalar_mul(bdtt[:], bdtt[:], 0.5)
        bdt2 = [bdtt[:, m:m + 1] for m in range(4)]

        # ---- load + transpose weights (emitted after chunk-0/1 stats so
        # the first x DMAs aren't queued behind 2.5MB of weight traffic) ----
        def emit_weights():
            # W_in [1024(feat), 256(d)] -> W_inT [2][128(d), 1024(feat)] * norm_w
            # One merged DMA per weight matrix: DMA triggers cost ~650ns of
            # serialized sequencer dispatch each.
            winr = wraw.tile([128, 8 * 256], f32, tag="winr", name="winr")
            nc.sync.dma_start(
                winr[:].rearrange("p (f d) -> p f d", d=256),
                win_d.rearrange("(f p) d -> p f d", p=128))
            win_raw = [winr[:, ts(f, 256)] for f in range(8)]
            winT = []
            for k in range(2):
                t = wT.tile([128, 1024], fr, tag=f"winT{k}", name=f"winT{k}")
                winT.append(t)
            for k in range(2):
                for half in range(2):
                    p = tps.tile([128, 512], f32, tag="tp", name="tpw")
                    for j in range(4):
                        f = half * 4 + j
                        nc.tensor.matmul(p[:, ts(j, 128)],
                                         win_raw[f][:, ts(k, 128)],
                                         identf[:], is_transpose=True)
                    if half % 2 == 0:
                        nc.vector.tensor_copy(winT[k][:, ts(half, 512)], p[:])
                    else:
                        nc.scalar.copy(winT[k][:, ts(half, 512)], p[:])
            for k in range(2):
                # fold norm_w (per-d row scale) into W_inT
                nc.vector.tensor_scalar_mul(winT[k][:], winT[k][:], nw[k])

            # W_dt [512(e_out), 512(e_in)] -> W_dtT [4][128(e_in), 512(e_out)]
            wdtr = wraw.tile([128, 4 * 512], f32, tag="wdtr", name="wdtr")
            nc.sync.dma_start(
                wdtr[:].rearrange("p (m e) -> p m e", e=512),
                wdt_d.rearrange("(m p) e -> p m e", p=128))
            wdt_raw = [wdtr[:, ts(m, 512)] for m in range(4)]
            wdtT = []
            for k in range(4):
                t = wT.tile([128, 512], fr, tag=f"wdtT{k}", name=f"wdtT{k}")
                wdtT.append(t)
            for k in range(4):
                p = tps.tile([128, 512], f32, tag="tp", name="tpw")
                for m in range(4):
                    nc.tensor.matmul(p[:, ts(m, 128)],
                                     wdt_raw[m][:, ts(k, 128)],
                                     identf[:], is_transpose=True)
                if k % 2 == 0:
                    nc.vector.tensor_copy(wdtT[k][:], p[:])
                else:
                    nc.scalar.copy(wdtT[k][:], p[:])

            # W_out [256(d), 512(e)] -> W_outT [4][128(e), 256(d)]
            woutr = wraw.tile([128, 2 * 512], f32, tag="woutr", name="woutr")
            nc.sync.dma_start(
                woutr[:].rearrange("p (dd e) -> p dd e", e=512),
                wout_d.rearrange("(dd p) e -> p dd e", p=128))
            wout_raw = [woutr[:, ts(dd, 512)] for dd in range(2)]
            woutT = []
            for e in range(4):
                t = wT.tile([128, 256], fr, tag=f"woutT{e}", name=f"woutT{e}")
                woutT.append(t)
            for e in range(4):
                p = tps.tile([128, 512], f32, tag="tp", name="tpw")
                for dd in range(2):
                    nc.tensor.matmul(p[:, ts(dd, 128)],
                                     wout_raw[dd][:, ts(e, 128)],
                                     identf[:], is_transpose=True)
                if e % 2 == 0:
                    nc.vector.tensor_copy(woutT[e][:], p[:, :256])
                else:
                    nc.scalar.copy(woutT[e][:], p[:, :256])
            return winT, wdtT, woutT

        # ---- main chunk loop ----
        def stats_stage(c):
            """Load x chunk, RMS stats -> inv_rms, bf16 x copy and
            diag(inv_rms) tiles (GpSimd). Returns state dict."""
            vpk = statp.tile([128, NTT], f32, tag="vpk", name="vpk")
            # two DMAs per chunk: merging to one serializes the transfer
            # on a single DGE queue on real hardware; four wastes sequencer
            # dispatch (~650ns each)
            xtile = xp.tile([128, NTT * D], f32, tag="xt", name="xt")
            for h in range(2):
                nc.sync.dma_start(
                    xtile[:, ts(h, NTT * D // 2)]
                    .rearrange("p (t d) -> p t d", d=D),
                    x_d[ts(c * 2 + h, TC // 2), :]
                    .rearrange("(t p) d -> p t d", p=128))
            xts = [xtile[:, ts(t, D)] for t in range(NTT)]
            for t in range(NTT):
                sq = sqp.tile([128, D], f32, tag="sq", name="sq")
                # fused square + row-sum (ScalarE). DVE tensor_tensor_reduce
                # would free the ScalarE but lowers to a custom DVE ucode op
                # that wedges the device under this runtime.
                nc.scalar.activation(sq[:], xts[t], AF.Square,
                                     accum_out=vpk[:, t:t + 1])

            # inv_rms = rsqrt(vpk/D + eps) via Newton on GpSimd (packed
            # [128,4]; tiny ops — keep them off the loaded DVE)
            nv = statp.tile([128, NTT], f32, tag="nv", name="nv")
            nc.gpsimd.tensor_scalar(nv[:], vpk[:], 1.0 / D, EPS,
                                    op0=ALU.mult, op1=ALU.add)
            ny = statp.tile([128, NTT], f32, tag="ny", name="ny")
            # seed: y0 = bits(magic - (bits(v) >> 1))
            nyi = ny[:].bitcast(i32)
            nc.vector.tensor_scalar(nyi, nv[:].bitcast(i32), 1, None,
                                    op0=ALU.arith_shift_right)
            nc.vector.scalar_tensor_tensor(nyi, magic[:], 1, nyi,
                                           op0=ALU.bypass, op1=ALU.subtract)
            nt = statp.tile([128, NTT], f32, tag="nt", name="nt")
            # 2 Newton iterations suffice: seed err ~3.4e-2 -> 1.7e-3 ->
            # 4.4e-6, far below the bf16 diag quantization (~4e-3)
            for _ in range(2):
                # t = v*y*y ; y = y * (1.5 - 0.5*t)
                nc.gpsimd.tensor_mul(nt[:], ny[:], ny[:])
                nc.gpsimd.tensor_mul(nt[:], nt[:], nv[:])
                nc.gpsimd.tensor_scalar(nt[:], nt[:], -0.5, 1.5,
                                        op0=ALU.mult, op1=ALU.add)
                nc.gpsimd.tensor_mul(ny[:], ny[:], nt[:])

            # bf16 x copy (transpose matmul operand) + diag(inv_rms) per
            # token-tile, both on GpSimd
            xbs, dgs = [], []
            for t in range(NTT):
                xb = xbp.tile([128, D], fr, tag="xb", name="xb")
                nc.gpsimd.tensor_copy(xb[:], xts[t])
                xbs.append(xb)
                dg = diagp.tile([128, 128], fr, tag="dg", name="dg")
                nc.gpsimd.tensor_scalar_mul(dg[:], identf[:], ny[:, t:t + 1])
                dgs.append(dg)
            return dict(xts=xts, xbs=xbs, dgs=dgs)

        def transp_stage(st):
            """hT[d, tok] = x[tok, d] * inv_rms[tok] via regular matmul:
            stationary = x-tile (bf16), moving = diag(inv_rms)."""
            xbs, dgs = st["xbs"], st["dgs"]
            hT = [hTp.tile([128, TC], fr, tag=f"hT{k}", name=f"hT{k}")
                  for k in range(2)]
            for k in range(2):
                p = tps.tile([128, TC], f32, tag="tp", name="tp")
                for t in range(NTT):
                    nc.tensor.matmul(p[:, ts(t, 128)], xbs[t][:, ts(k, 128)],
                                     dgs[t][:])
                nc.vector.tensor_copy(hT[k][:], p[:])
            st["hT"] = hT

        def in_front_stage(st):
            """in_proj + silu: only needs hT, so it can be emitted a full
            chunk earlier than the dt/scan work (3-deep pipeline)."""
            hT = st["hT"]
            uT = [uTp.tile([128, TC], fr, tag=f"uT{m}", name=f"uT{m}")
                  for m in range(4)]
            gT = [gTp.tile([128, TC], fr, tag=f"gT{m}", name=f"gT{m}")
                  for m in range(4)]
            for m in range(8):
                ps = uvps.tile([128, TC], f32, tag="uv", name="uv")
                for k in range(2):
                    nc.tensor.matmul(
                        ps[:], winT[k][:, ts(m, 128)], hT[k][:],
                        start=(k == 0), stop=(k == 1),
                    )
                dst = uT[m] if m < 4 else gT[m - 4]
                if silu_native:
                    nc.scalar.activation(dst[:], ps[:], AF.Silu)
                else:
                    # CoreSim has no Silu: decompose as x * sigmoid(x)
                    sig = sqp.tile([128, TC], f32, tag="sig", name="sig")
                    nc.scalar.activation(sig[:], ps[:], AF.Sigmoid)
                    nc.vector.tensor_mul(dst[:], ps[:], sig[:])
            st.update(uT=uT, gT=gT)

        def dt_front_stage(st):
            """dt_proj + tanh + lam + bT — the pre-scan work."""
            uT = st["uT"]
            thT = [thTp.tile([128, TC], fr, tag=f"thT{m}", name=f"thT{m}")
                   for m in range(4)]
            lamT = [lamTp.tile([128, TC], fr, tag=f"lamT{m}", name=f"lamT{m}")
                    for m in range(4)]
            bT = [bTp.tile([128, TC], fr, tag=f"bT{m}", name=f"bT{m}")
                  for m in range(4)]
            bt_eng = nc.gpsimd if bt_on_pool else nc.vector
            for m in range(4):
                ps = preps.tile([128, TC], f32, tag="pre", name="pre")
                for k in range(4):
                    nc.tensor.matmul(
                        ps[:], wdtT[k][:, ts(m, 128)], uT[k][:],
                        start=(k == 0), stop=(k == 3),
                    )
                nc.scalar.activation(thT[m][:], ps[:], AF.Tanh,
                                     bias=bdt2[m], scale=0.5)
                # lam = sigmoid(pre + b_dt) = 0.5*th + 0.5 (DVE 4x mode)
                nc.vector.tensor_scalar(lamT[m][:], thT[m][:], 0.5, 0.5,
                                        op0=ALU.mult, op1=ALU.add)
                # bT = (th - 1) * u, one fused op
                bt_eng.scalar_tensor_tensor(bT[m][:], thT[m][:], 1.0, uT[m][:],
                                            op0=ALU.subtract, op1=ALU.mult)
            st.update(lamT=lamT, bT=bT)

        def scan_stage(st, sT_prev):
            # scan: S_t = lam_t*S_{t-1} + bT_t  => S = -2*s
            sT = [sTp.tile([128, TC], fr, tag=f"sT{m}", name=f"sT{m}")
                  for m in range(4)]
            sgT = [sgTp.tile([128, TC], fr, tag=f"sgT{m}", name=f"sgT{m}")
                   for m in range(4)]
            sg_eng = nc.gpsimd if sg_on_pool else nc.vector
            for m in range(4):
                scan_eng = nc.gpsimd if m < scan_on_pool else nc.vector
                init = 0.0 if sT_prev is None else sT_prev[m][:, TC - 1:TC]
                scan_eng.tensor_tensor_scan(
                    sT[m][:], st["lamT"][m][:], st["bT"][m][:], init,
                    op0=ALU.mult, op1=ALU.add,
                )
                # sg = s*g = (S * -0.5) * g, right after its scan so
                # out_proj's e-th accumulation can start immediately
                sg_eng.scalar_tensor_tensor(
                    sgT[m][:], sT[m][:], -0.5, st["gT"][m][:],
                    op0=ALU.mult, op1=ALU.mult,
                )
            st.update(sT=sT, sgT=sgT)
            return sT

        def out_stage(st, c):
            # out_proj (sgT stationary -> y in [tok, d]); the residual is a
            # 5th accumulation matmul (identity stationary, bf16 x moving),
            # so PSUM holds x+y and the drain is a plain ScalarE copy
            # (PSUM-source, native speed) instead of a DVE add.
            sgT, xbs = st["sgT"], st["xbs"]
            for pair in range(NTT // 2):
                tt = (2 * pair, 2 * pair + 1)
                yp2 = [yps.tile([128, D], f32, tag="y", name="y") for _ in tt]
                for e in range(4):
                    for i, t in enumerate(tt):
                        nc.tensor.matmul(
                            yp2[i][:], sgT[e][:, ts(t, 128)], woutT[e][:],
                            start=(e == 0), stop=False,
                        )
                ot = outp.tile([128, 2 * D], f32, tag="ot", name="ot")
                for i, t in enumerate(tt):
                    nc.tensor.matmul(
                        yp2[i][:], identb[:], xbs[t][:],
                        start=False, stop=True,
                    )
                    if i == 0:
                        nc.vector.tensor_copy(ot[:, ts(i, D)], yp2[i][:])
                    else:
                        nc.scalar.copy(ot[:, ts(i, D)], yp2[i][:])
                # one paired store
                nc.sync.dma_start(
                    out_d[ts(c * 2 + pair, 256), :]
                    .rearrange("(i p) d -> p i d", p=128),
                    ot[:].rearrange("p (i d) -> p i d", d=D))

        # chunk-0 stats first (x0 DMA leads the queue), then weights, then
        # chunk-1 stats (x1 DMA arrives within the first chunk period)
        pre0 = stats_stage(0)
        winT, wdtT, woutT = emit_weights()
        pre_sts = (pre0, stats_stage(1))

        # Software-pipelined emission. Engines run their streams in order,
        # so next chunk's PE-heavy front must be emitted BEFORE this
        # chunk's out_proj for PE to stay busy during the scan tail; the
        # c+2 transposes go last so the PE reaches them only after the
        # out_proj matmuls.
        for r in range(repeat):
            sts = [dict() for _ in range(NCHUNK)]
            if r == 0:
                # first iteration: chunk-0/1 stats were emitted before the
                # weight prep so their DMAs aren't queued behind 2.5MB of
                # weight traffic
                sts[0], sts[1] = pre_sts
            else:
                sts[0] = stats_stage(0)
                if NCHUNK > 1:
                    sts[1] = stats_stage(1)
            transp_stage(sts[0])
            in_front_stage(sts[0])
            if NCHUNK > 1:
                transp_stage(sts[1])
                dt_front_stage(sts[0])
                in_front_stage(sts[1])
            else:
                dt_front_stage(sts[0])
            sT_prev = None
            for c in range(NCHUNK):
                sT_prev = scan_stage(sts[c], sT_prev)
                if c + 1 < NCHUNK:
                    dt_front_stage(sts[c + 1])
                if c + 2 < NCHUNK:
                    sts[c + 2] = stats_stage(c + 2)
                    transp_stage(sts[c + 2])
                out_stage(sts[c], c)
                if c + 2 < NCHUNK:
                    in_front_stage(sts[c + 2])
                sts[c].clear()


def build(mm_f32r=True, silu_native=True, repeat=1, **flags):
    """Build and compile the Bass module (cached)."""
    key = ("nc", mm_f32r, silu_native, repeat, tuple(sorted(flags.items())))
    if key in _CACHE:
        return _CACHE[key]

    from concourse import bacc, mybir, tile

    f32 = mybir.dt.float32
    nc = bacc.Bacc(
        "TRN2",
        target_bir_lowering=False,
        debug=False,
        num_devices=N_CORES,
    )
    aps = {
        "x": nc.dram_tensor("x", [K, D], f32, kind="ExternalInput").ap(),
        "norm_w": nc.dram_tensor("norm_w", [D], f32, kind="ExternalInput").ap(),
        "W_in": nc.dram_tensor("W_in", [2 * INNER, D], f32, kind="ExternalInput").ap(),
        "W_dt": nc.dram_tensor("W_dt", [INNER, INNER], f32, kind="ExternalInput").ap(),
        "b_dt": nc.dram_tensor("b_dt", [INNER], f32, kind="ExternalInput").ap(),
        "W_out": nc.dram_tensor("W_out", [D, INNER], f32, kind="ExternalInput").ap(),
        "out": nc.dram_tensor("out", [K, D], f32, kind="ExternalOutput").ap(),
    }
    with tile.TileContext(nc) as tc:
        _emit(tc, aps, mm_f32r=mm_f32r, silu_native=silu_native,
              repeat=repeat, **flags)
    nc.compile()
    _CACHE[key] = nc
    return nc


def make_in_maps(inputs):
    x = np.asarray(inputs["x"], dtype=np.float32)
    shared = {
        "norm_w": np.asarray(inputs["norm_w"], dtype=np.float32),
        "W_in": np.asarray(inputs["W_in"], dtype=np.float32),
        "W_dt": np.asarray(inputs["W_dt"], dtype=np.float32),
        "b_dt": np.asarray(inputs["b_dt"], dtype=np.float32),
        "W_out": np.asarray(inputs["W_out"], dtype=np.float32),
    }
    return [
        {"x": np.ascontiguousarray(x[b]), **shared} for b in range(N_CORES)
    ]


def run(inputs, trace=False, mm_f32r=True, silu_native=True, **kw):
    from concourse.bass_utils import run_bass_kernel_spmd

    nc = build(mm_f32r=mm_f32r, silu_native=silu_native)
    in_maps = make_in_maps(inputs)
    res = run_bass_kernel_spmd(
        nc, in_maps, core_ids=list(range(N_CORES)), trace=trace, **kw
    )
    out = np.stack([res.results[b]["out"] for b in range(N_CORES)], axis=0)
    return out, res


def kernel(**inputs) -> np.ndarray:
    out, _ = run(inputs, trace=False)
    return out
